# revision 25
# baseline (speedup 1.0000x reference)
"""Trainium2 Bass kernel for a 12-layer EVA-style ViT encoder (B=16, N=256, D=768).

Sharding: pure data-parallel over batch across 8 NeuronCores (2 images/core).
Per core: feature-major activations [feature, token] (T=512 token columns).
v2: fp16 matmuls (1 cycle/row vs f32r's 1.5), replicated LN stats (no serial
[1,T] DVE chain), softmax denominator fused into the PV matmul via a ones
column in V, batched normalization, fp32 residual stream, preloaded weights
with single large DMAs per matrix.
"""
import sys, types

sys.path.insert(0, '/opt/trn_rl_repo')

import numpy as np

B, NTOK, DIM, HEADS, HD, DEPTH, HIDDEN = 16, 256, 768, 12, 64, 12, 2048
EPS = 1e-5
NCORES = 8
BPC = B // NCORES          # batch items per core
T = BPC * NTOK             # 512 token columns per core
KD = DIM // 128            # 6
KH = HIDDEN // 128         # 16
SCALE = HD ** -0.5

_CACHE = {}


def _install_ntff_shim():
    if "antenv.axon_hooks" in sys.modules:
        return
    m = types.ModuleType("antenv.axon_hooks")
    m._hook = None
    m.set_axon_ntff_profile_hook = lambda h: setattr(m, "_hook", h)
    m.get_axon_ntff_profile_hook = lambda: m._hook
    sys.modules["antenv.axon_hooks"] = m
    try:
        from trn_agent_boot.trn_boot import _ntff_profile_via_ctypes
        m.set_axon_ntff_profile_hook(_ntff_profile_via_ctypes('/opt/axon/libaxon_pjrt.so'))
    except Exception:
        pass


def _build(layers=DEPTH, dbg=False):
    import concourse.bass as bass
    import concourse.mybir as mybir
    import concourse.tile as tile
    from concourse import bacc
    from contextlib import ExitStack

    f32 = mybir.dt.float32
    f16 = mybir.dt.float16
    AF = mybir.ActivationFunctionType
    OP = mybir.AluOpType

    nc = bacc.Bacc("TRN2", target_bir_lowering=False, debug=False)

    x_fm = nc.dram_tensor("x_fm", [DIM, T], f32, kind="ExternalInput")
    # weights, partition-major fp16: [L, 128, ktiles*outcols]
    WQ = nc.dram_tensor("WQ", [layers, 128, KD * DIM], f16, kind="ExternalInput")
    WK = nc.dram_tensor("WK", [layers, 128, KD * DIM], f16, kind="ExternalInput")
    WV = nc.dram_tensor("WV", [layers, 128, KD * DIM], f16, kind="ExternalInput")
    WO = nc.dram_tensor("WO", [layers, 128, KD * DIM], f16, kind="ExternalInput")
    W1G = nc.dram_tensor("W1G", [layers, 4, 128, KD * 512], f16, kind="ExternalInput")
    W1X = nc.dram_tensor("W1X", [layers, 4, 128, KD * 512], f16, kind="ExternalInput")
    W2 = nc.dram_tensor("W2", [layers, 128, KH * DIM], f16, kind="ExternalInput")
    SIN = nc.dram_tensor("SIN", [128, T], f16, kind="ExternalInput")
    COS = nc.dram_tensor("COS", [128, T], f16, kind="ExternalInput")
    PERM = nc.dram_tensor("PERM", [128, 128], f16, kind="ExternalInput")
    ONES = nc.dram_tensor("ONES", [128, 128], f16, kind="ExternalInput")
    REP2 = nc.dram_tensor("REP2", [33, 128], f16, kind="ExternalInput")
    out_fm = nc.dram_tensor("out_fm", [DIM, T], f32, kind="ExternalOutput")
    if dbg:
        DY = nc.dram_tensor("DY", [DIM, T], f16, kind="ExternalOutput")
        DQ = nc.dram_tensor("DQ", [DIM, T], f16, kind="ExternalOutput")
        DK = nc.dram_tensor("DK", [DIM, T], f16, kind="ExternalOutput")
        DV = nc.dram_tensor("DV", [512, 12 * 65], f16, kind="ExternalOutput")
        DE = nc.dram_tensor("DE", [128, 512], f16, kind="ExternalOutput")
        DOT = nc.dram_tensor("DOT", [DIM, T], f16, kind="ExternalOutput")
        DH = nc.dram_tensor("DH", [DIM, T], f32, kind="ExternalOutput")
        DS = nc.dram_tensor("DS", [HIDDEN, T], f16, kind="ExternalOutput")

    with tile.TileContext(nc) as tc:
        with ExitStack() as ctx:
            ctx.enter_context(nc.allow_low_precision(
                reason="fp16 matmul inputs, fp32 psum accumulation and residual"))
            const = ctx.enter_context(tc.tile_pool(name="const", bufs=1))
            hp = ctx.enter_context(tc.tile_pool(name="hp", bufs=1))
            stp = ctx.enter_context(tc.tile_pool(name="stp", bufs=1))   # stats f16 tiles
            lnp = ctx.enter_context(tc.tile_pool(name="lnp", bufs=1))   # LN chain tiles
            yp = ctx.enter_context(tc.tile_pool(name="yp", bufs=1))
            qrawp = ctx.enter_context(tc.tile_pool(name="qrawp", bufs=1))
            ropep = ctx.enter_context(tc.tile_pool(name="ropep", bufs=1))
            rtmp = ctx.enter_context(tc.tile_pool(name="rtmp", bufs=2))
            vp = ctx.enter_context(tc.tile_pool(name="vp", bufs=1))
            ep = ctx.enter_context(tc.tile_pool(name="ep", bufs=1))
            dnp = ctx.enter_context(tc.tile_pool(name="dnp", bufs=1))
            op_ = ctx.enter_context(tc.tile_pool(name="op", bufs=1))
            sp = ctx.enter_context(tc.tile_pool(name="sp", bufs=1))
            snp = ctx.enter_context(tc.tile_pool(name="snp", bufs=1))
            wqkp = ctx.enter_context(tc.tile_pool(name="wqkp", bufs=1))
            wvop = ctx.enter_context(tc.tile_pool(name="wvop", bufs=1))
            wmp = ctx.enter_context(tc.tile_pool(name="wmp", bufs=2))
            w2p = ctx.enter_context(tc.tile_pool(name="w2p", bufs=1))
            PS = ctx.enter_context(tc.tile_pool(name="PS", bufs=1, space="PSUM"))

            def pst(tag, shape=None, name=None):
                return PS.tile(shape or [128, T], f32,
                               name=name or f"ps_{tag}_{nc.next_id()}", tag=tag)

            # ---------------- constants ----------------
            ones16 = const.tile([128, 128], f16)
            nc.sync.dma_start(out=ones16, in_=ONES[:, :])
            sin16 = const.tile([128, T], f16)
            nc.sync.dma_start(out=sin16, in_=SIN[:, :])
            cos16 = const.tile([128, T], f16)
            nc.sync.dma_start(out=cos16, in_=COS[:, :])
            perm16 = const.tile([128, 128], f16)
            nc.sync.dma_start(out=perm16, in_=PERM[:, :])
            rep2 = const.tile([33, 128], f16)
            nc.sync.dma_start(out=rep2, in_=REP2[:, :])
            epsc = const.tile([128, 1], f32)
            nc.vector.memset(epsc, EPS)
            warm8 = const.tile([128, 8], f16)
            nc.vector.memset(warm8, 1.0)

            # PE warmup
            wps = pst("p7", [8, 8], name="warmps")
            nc.tensor.matmul(wps, warm8[:, :], warm8[:, 0:8], start=True, stop=True)

            # V tiles (token-major, 12 heads x (64 cols + 1 ones col))
            vtm = []
            for mt in range(4):
                v_ = vp.tile([128, 12, 65], f16, name=f"vtm{mt}", tag=f"v{mt}")
                nc.vector.memset(v_[:, :, 64:65], 1.0)
                vtm.append(v_)
            # softmax denominator tiles: rows 0 and 32 hold the two heads of a
            # group (32-aligned partition shifts from PSUM row 64); other rows
            # stay 1.0 so Ln/Exp of them is benign
            den65 = dnp.tile([33, T], f32, name="den65", tag="den")
            nc.vector.memset(den65, 1.0)
            lnd65 = dnp.tile([33, T], f32, name="lnd65", tag="lnd")
            rec65 = dnp.tile([33, T], f16, name="rec65", tag="rec")

            h = []
            for k in range(KD):
                t_ = hp.tile([128, T], f32, name=f"h_{k}", tag=f"h{k}")
                nc.sync.dma_start(out=t_, in_=x_fm[128 * k:128 * (k + 1), :])
                h.append(t_)

            def layer_norm(tag, src_tiles, D, n_src):
                """Replicated-stats LN. src_tiles: list of [128,T] f32 tiles (or f16).
                Returns (A16, bneg16): replicated [128,T] f16 rstd and -mean."""
                inv = 1.0 / D
                srowR = pst("p6", name=f"srow_{tag}")
                qrowR = pst("p7", name=f"qrow_{tag}")
                for k in range(n_src):
                    h16 = stp.tile([128, T], f16, name=f"h16_{tag}_{k}", tag=f"st{k % 3}")
                    nc.scalar.copy(h16[:, :], src_tiles[k][:, :])
                    sq16 = stp.tile([128, T], f16, name=f"sq16_{tag}_{k}", tag=f"sq{k % 3}")
                    nc.scalar.activation(sq16[:, :], src_tiles[k][:, :], AF.Square)
                    nc.tensor.matmul(srowR, ones16[:, :], h16[:, :],
                                     start=(k == 0), stop=(k == n_src - 1))
                    nc.tensor.matmul(qrowR, ones16[:, :], sq16[:, :],
                                     start=(k == 0), stop=(k == n_src - 1))
                bneg16 = lnp.tile([128, T], f16, name=f"bneg_{tag}", tag="bneg")
                nc.vector.tensor_scalar(bneg16[:, :], srowR[:, :], -inv, None, op0=OP.mult)
                # keep the PE activity window alive through the DVE chain
                bl = pst("p6", [8, 8], name=f"blip_{tag}")
                nc.tensor.matmul(bl, warm8[:, 0:8], bneg16[:, 0:8], start=True, stop=True)
                msq = lnp.tile([128, T], f32, name=f"msq_{tag}", tag="msq")
                nc.vector.tensor_mul(msq[:, :], bneg16[:, :], bneg16[:, :])
                ve = lnp.tile([128, T], f32, name=f"ve_{tag}", tag="ve")
                nc.vector.scalar_tensor_tensor(ve[:, :], qrowR[:, :], inv, msq[:, :],
                                               op0=OP.mult, op1=OP.subtract)
                rv = lnp.tile([128, T], f32, name=f"rv_{tag}", tag="rv")
                nc.vector.reciprocal_approx_fast(out=rv[:, :], in_=ve[:, :])
                A16 = lnp.tile([128, T], f16, name=f"A_{tag}", tag="A16")
                nc.scalar.activation(A16[:, :], rv[:, :], AF.Sqrt)
                return A16, bneg16

            def ln_apply(tag, k, src, A16, bneg16, out_pool, out_tag):
                tmp = rtmp.tile([128, T], f16, name=f"lt_{tag}_{k}", tag="lnt")
                nc.vector.tensor_add(tmp[:, :], src[:, :], bneg16[:, :])
                y_ = out_pool.tile([128, T], f16, name=f"y_{tag}_{k}", tag=out_tag)
                nc.vector.tensor_mul(y_[:, :], tmp[:, :], A16[:, :])
                return y_

            for l in range(layers):
                # prefetch W2 for this layer early (used last)
                w2_all = w2p.tile([128, KH, DIM], f16, name=f"w2_{l}", tag="w2")
                nc.sync.dma_start(out=w2_all, in_=W2[l, :, :])

                # ---------------- LN1 ----------------
                A1, B1 = layer_norm(f"l1_{l}", h, DIM, KD)
                y1 = [ln_apply(f"y1_{l}", k, h[k], A1, B1, yp, f"y{k}")
                      for k in range(KD)]

                # ---------------- Q,K projections + RoPE (m-outer) ----------------
                wq_all = wqkp.tile([128, KD, DIM], f16, name=f"wq_{l}", tag="wq")
                nc.sync.dma_start(out=wq_all, in_=WQ[l, :, :])
                wk_all = wqkp.tile([128, KD, DIM], f16, name=f"wk_{l}", tag="wk")
                nc.sync.dma_start(out=wk_all, in_=WK[l, :, :])
                QK_TAGS = ["p0", "p1", "p2", "p3"]
                ROT_TAGS = ["p4", "p5"]
                qs, ks = [], []
                for m in range(KD):
                    # Q and K for the same m interleaved across two PSUM banks
                    psq = pst(QK_TAGS[(2 * m) % 4], name=f"psq{l}_{m}")
                    psk = pst(QK_TAGS[(2 * m + 1) % 4], name=f"psk{l}_{m}")
                    for k in range(KD):
                        nc.tensor.matmul(psq, wq_all[:, k, 128 * m:128 * (m + 1)],
                                         y1[k][:, :], start=(k == 0), stop=(k == KD - 1))
                        nc.tensor.matmul(psk, wk_all[:, k, 128 * m:128 * (m + 1)],
                                         y1[k][:, :], start=(k == 0), stop=(k == KD - 1))
                    for wname, ps, outs in (("q", psq, qs), ("k", psk, ks)):
                        idx = (0 if wname == "q" else 1) + 2 * m
                        raw = qrawp.tile([128, T], f16, name=f"{wname}raw{l}_{m}",
                                         tag=f"qr{idx % 2}")
                        nc.scalar.copy(raw[:, :], ps[:, :])
                        rot = pst(ROT_TAGS[idx % 2], name=f"rot{wname}{l}_{m}")
                        nc.tensor.matmul(rot, perm16[:, :], raw[:, :], start=True, stop=True)
                        t1 = rtmp.tile([128, T], f16, name=f"t1{wname}{l}_{m}", tag="t1")
                        nc.vector.tensor_mul(t1[:, :], rot[:, :], sin16[:, :])
                        t2 = rtmp.tile([128, T], f16, name=f"t2{wname}{l}_{m}", tag="t2")
                        nc.vector.tensor_mul(t2[:, :], raw[:, :], cos16[:, :])
                        rp = ropep.tile([128, T], f16, name=f"{wname}p{l}_{m}",
                                        tag=f"{wname}p{m}")
                        nc.vector.tensor_add(rp[:, :], t1[:, :], t2[:, :])
                        outs.append(rp)

                # ---------------- V projection (token-major, ones col persists) ----
                wv_all = wvop.tile([128, KD, DIM], f16, name=f"wv_{l}", tag="wv")
                nc.sync.dma_start(out=wv_all, in_=WV[l, :, :])
                V_TAGS = ["p0", "p1", "p2", "p3"]
                for mt in range(4):
                    psvA = pst(V_TAGS[(2 * mt) % 4], [128, 6, 64], name=f"psvA{l}_{mt}")
                    psvB = pst(V_TAGS[(2 * mt + 1) % 4], [128, 6, 64], name=f"psvB{l}_{mt}")
                    for k in range(KD):
                        nc.tensor.matmul(psvA, y1[k][:, 128 * mt:128 * (mt + 1)],
                                         wv_all[:, k, 0:384], start=(k == 0), stop=(k == KD - 1))
                        nc.tensor.matmul(psvB, y1[k][:, 128 * mt:128 * (mt + 1)],
                                         wv_all[:, k, 384:768], start=(k == 0), stop=(k == KD - 1))
                    # scatter heads into 65-stride slots (col 64 of each head = ones)
                    nc.scalar.copy(vtm[mt][:, 0:6, 0:64], psvA[:, :, :])
                    nc.scalar.copy(vtm[mt][:, 6:12, 0:64], psvB[:, :, :])

                if dbg and l == 0:
                    for k in range(KD):
                        nc.sync.dma_start(out=DY[128 * k:128 * (k + 1), :], in_=y1[k][:, :])
                        nc.sync.dma_start(out=DQ[128 * k:128 * (k + 1), :], in_=qs[k][:, :])
                        nc.sync.dma_start(out=DK[128 * k:128 * (k + 1), :], in_=ks[k][:, :])
                    for mt in range(4):
                        nc.sync.dma_start(out=DV[128 * mt:128 * (mt + 1), :],
                                          in_=vtm[mt][:, :, :])

                # ---------------- attention ----------------
                o16 = [op_.tile([128, T], f16, name=f"o16_{l}_{p}", tag=f"o{p}")
                       for p in range(KD)]
                for p in range(KD):          # head-pair groups
                    psO = {}
                    PO_TAGS = ["p2", "p3"] if p % 2 == 0 else ["p0", "p1"]
                    for jj, (hh, i) in enumerate(
                            [(2 * p, 0), (2 * p, 1), (2 * p + 1, 0), (2 * p + 1, 1)]):
                        off = 64 * (hh % 2)
                        psS = pst(["p4", "p5"][jj % 2], name=f"psS{l}_{p}_{jj}")
                        for kt in range(2):
                            nc.tensor.matmul(
                                psS[:, NTOK * kt:NTOK * (kt + 1)],
                                ks[p][off:off + 64,
                                      256 * i + 128 * kt:256 * i + 128 * (kt + 1)],
                                qs[p][off:off + 64, 256 * i:256 * (i + 1)],
                                start=True, stop=True)
                        eT = ep.tile([128, 2 * NTOK], f16, name=f"eT{l}_{p}_{jj}",
                                     tag=f"eT{jj % 2}")
                        nc.scalar.activation(eT[:, :], psS[:, :], AF.Exp)
                        if dbg and l == 0 and p == 0 and jj == 0:
                            nc.sync.dma_start(out=DE[:, :], in_=eT[:, :])
                        if hh not in psO:
                            psO[hh] = pst(PO_TAGS[hh % 2], [65, 2 * NTOK],
                                          name=f"psO{l}_{p}_{hh}")
                        for kt in range(2):
                            nc.tensor.matmul(psO[hh][:, NTOK * i:NTOK * (i + 1)],
                                             vtm[2 * i + kt][:, hh, :],
                                             eT[:, NTOK * kt:NTOK * (kt + 1)],
                                             start=(kt == 0), stop=(kt == 1))
                        r0 = 32 * (hh % 2)
                        nc.scalar.copy(den65[r0:r0 + 1, NTOK * i:NTOK * (i + 1)],
                                       psO[hh][64:65, NTOK * i:NTOK * (i + 1)])
                    # rec = 1/den via fast-approx reciprocal on DVE (no act table)
                    nc.vector.reciprocal_approx_fast(out=lnd65[:, :], in_=den65[:, :])
                    nc.scalar.copy(rec65[:, :], lnd65[:, :])
                    psR = pst(["p6", "p7"][p % 2], name=f"psR{l}_{p}")
                    nc.tensor.matmul(psR, rep2[:, :], rec65[:, :], start=True, stop=True)
                    Rec16 = dnp.tile([128, T], f16, name=f"Rec{l}_{p}", tag="Rec")
                    nc.scalar.copy(Rec16[:, :], psR[:, :])
                    for hh in (2 * p, 2 * p + 1):
                        off = 64 * (hh % 2)
                        nc.vector.tensor_mul(o16[p][off:off + 64, :],
                                             psO[hh][0:64, :], Rec16[off:off + 64, :])

                # ---------------- O projection + residual ----------------
                wo_all = wvop.tile([128, KD, DIM], f16, name=f"wo_{l}", tag="wo")
                nc.sync.dma_start(out=wo_all, in_=WO[l, :, :])
                O_TAGS = ["p5", "p6", "p7", "p0"]
                for m0 in range(0, KD, 2):
                    psa = pst(O_TAGS[m0 % 4], name=f"psh{l}_{m0}")
                    psb = pst(O_TAGS[(m0 + 1) % 4], name=f"psh{l}_{m0 + 1}")
                    for k in range(KD):
                        nc.tensor.matmul(psa, wo_all[:, k, 128 * m0:128 * (m0 + 1)],
                                         o16[k][:, :], start=(k == 0), stop=(k == KD - 1))
                        nc.tensor.matmul(psb, wo_all[:, k, 128 * (m0 + 1):128 * (m0 + 2)],
                                         o16[k][:, :], start=(k == 0), stop=(k == KD - 1))
                    nc.vector.tensor_add(h[m0][:, :], h[m0][:, :], psa[:, :])
                    nc.vector.tensor_add(h[m0 + 1][:, :], h[m0 + 1][:, :], psb[:, :])

                if dbg and l == 0:
                    for k in range(KD):
                        nc.sync.dma_start(out=DOT[128 * k:128 * (k + 1), :], in_=o16[k][:, :])
                        nc.sync.dma_start(out=DH[128 * k:128 * (k + 1), :], in_=h[k][:, :])

                # ---------------- LN2 ----------------
                A2, B2 = layer_norm(f"l2_{l}", h, DIM, KD)
                y2 = [ln_apply(f"y2_{l}", k, h[k], A2, B2, yp, f"y{k}")
                      for k in range(KD)]

                # ---------------- MLP G/U (chunks of 128 hidden) ----------------
                srow2 = pst("p6", name=f"srowm_{l}")
                qrow2 = pst("p7", name=f"qrowm_{l}")
                G_TAGS = ["p1", "p2", "p3"]
                U_TAGS = ["p4", "p5", "p0"]
                s_list = []
                for c4 in range(4):
                    wg4 = wmp.tile([128, KD, 512], f16, name=f"wg{l}_{c4}", tag="wg")
                    nc.sync.dma_start(out=wg4, in_=W1G[l, c4, :, :])
                    wx4 = wmp.tile([128, KD, 512], f16, name=f"wx{l}_{c4}", tag="wx")
                    nc.sync.dma_start(out=wx4, in_=W1X[l, c4, :, :])
                    for cc in range(4):
                        c = 4 * c4 + cc
                        psG = pst(G_TAGS[c % 3], name=f"psG{l}_{c}")
                        psU = pst(U_TAGS[c % 3], name=f"psU{l}_{c}")
                        for k in range(KD):
                            nc.tensor.matmul(psG, wg4[:, k, 128 * cc:128 * (cc + 1)],
                                             y2[k][:, :], start=(k == 0), stop=(k == KD - 1))
                            nc.tensor.matmul(psU, wx4[:, k, 128 * cc:128 * (cc + 1)],
                                             y2[k][:, :], start=(k == 0), stop=(k == KD - 1))
                        # 2*silu(g)*u = (tanh(g/2)+1)*(g*u); the factor of 2 is
                        # washed out by the inner LayerNorm
                        th = sp.tile([128, T], f16, name=f"th{l}_{c}", tag=f"th{c % 2}")
                        nc.scalar.activation(th[:, :], psG[:, :], AF.Tanh, scale=0.5)
                        uc = sp.tile([128, T], f16, name=f"uc{l}_{c}", tag=f"uc{c % 2}")
                        nc.scalar.copy(uc[:, :], psU[:, :])
                        pp = sp.tile([128, T], f16, name=f"pp{l}_{c}", tag=f"pp{c % 2}")
                        nc.vector.tensor_mul(pp[:, :], psG[:, :], uc[:, :])
                        s_ = snp.tile([128, T], f16, name=f"s{l}_{c}", tag=f"s{c}")
                        nc.vector.scalar_tensor_tensor(s_[:, :], th[:, :], 1.0, pp[:, :],
                                                       op0=OP.add, op1=OP.mult)
                        sq_ = stp.tile([128, T], f16, name=f"ssq{l}_{c}", tag=f"sq{c % 3}")
                        nc.scalar.activation(sq_[:, :], s_[:, :], AF.Square)
                        nc.tensor.matmul(srow2, ones16[:, :], s_[:, :],
                                         start=(c == 0), stop=(c == KH - 1))
                        nc.tensor.matmul(qrow2, ones16[:, :], sq_[:, :],
                                         start=(c == 0), stop=(c == KH - 1))
                        s_list.append(s_)

                if dbg and l == 0:
                    for c in range(KH):
                        nc.sync.dma_start(out=DS[128 * c:128 * (c + 1), :], in_=s_list[c][:, :])

                # ---------------- MLP LN ----------------
                inv = 1.0 / HIDDEN
                bnegm = lnp.tile([128, T], f16, name=f"bnegm_{l}", tag="bneg")
                nc.vector.tensor_scalar(bnegm[:, :], srow2[:, :], -inv, None, op0=OP.mult)
                blm = pst("p6", [8, 8], name=f"blipm_{l}")
                nc.tensor.matmul(blm, warm8[:, 0:8], bnegm[:, 0:8], start=True, stop=True)
                msqm = lnp.tile([128, T], f32, name=f"msqm_{l}", tag="msq")
                nc.vector.tensor_mul(msqm[:, :], bnegm[:, :], bnegm[:, :])
                vem = lnp.tile([128, T], f32, name=f"vem_{l}", tag="ve")
                nc.vector.scalar_tensor_tensor(vem[:, :], qrow2[:, :], inv, msqm[:, :],
                                               op0=OP.mult, op1=OP.subtract)
                rvm = lnp.tile([128, T], f32, name=f"rvm_{l}", tag="rv")
                nc.vector.reciprocal_approx_fast(out=rvm[:, :], in_=vem[:, :])
                Am = lnp.tile([128, T], f16, name=f"Am_{l}", tag="A16")
                nc.scalar.activation(Am[:, :], rvm[:, :], AF.Sqrt)
                mN = []
                for c in range(KH):
                    tmp = rtmp.tile([128, T], f16, name=f"mt{l}_{c}", tag="lnt")
                    nc.vector.tensor_add(tmp[:, :], s_list[c][:, :], bnegm[:, :])
                    nc.vector.tensor_mul(s_list[c][:, :], tmp[:, :], Am[:, :])
                    mN.append(s_list[c])

                # ---------------- W2 + residual ----------------
                W2_TAGS = ["p1", "p2", "p3", "p4"]
                for m0 in range(0, KD, 2):
                    psa = pst(W2_TAGS[m0 % 4], name=f"psm{l}_{m0}")
                    psb = pst(W2_TAGS[(m0 + 1) % 4], name=f"psm{l}_{m0 + 1}")
                    for k in range(KH):
                        nc.tensor.matmul(psa, w2_all[:, k, 128 * m0:128 * (m0 + 1)],
                                         mN[k][:, :], start=(k == 0), stop=(k == KH - 1))
                        nc.tensor.matmul(psb, w2_all[:, k, 128 * (m0 + 1):128 * (m0 + 2)],
                                         mN[k][:, :], start=(k == 0), stop=(k == KH - 1))
                    nc.vector.tensor_add(h[m0][:, :], h[m0][:, :], psa[:, :])
                    nc.vector.tensor_add(h[m0 + 1][:, :], h[m0 + 1][:, :], psb[:, :])

            for k in range(KD):
                nc.sync.dma_start(out=out_fm[128 * k:128 * (k + 1), :],
                                  in_=h[k][:, :])

    nc.compile()
    return nc


def _prep_host(inputs, layers=DEPTH):
    x = np.asarray(inputs['x'], np.float32)
    pos = np.asarray(inputs['pos_embed'], np.float32)
    rope = np.asarray(inputs['rope_emb'], np.float32)
    g = lambda n: np.asarray(inputs[n], np.float32)

    for n in ('bq', 'bv', 'bo', 'b1g', 'b1x', 'b2', 'ln1_b', 'ln2_b', 'lnm_b'):
        assert np.abs(g(n)).max() == 0.0, f"nonzero bias {n} unsupported"

    ln1w, ln2w, lnmw = g('ln1_w'), g('ln2_w'), g('lnm_w')
    wq = g('wq') * ln1w[:, None, :] * SCALE
    wk = g('wk') * ln1w[:, None, :]
    wv = g('wv') * ln1w[:, None, :]
    wo = g('wo')
    w1g = g('w1g') * ln2w[:, None, :]
    w1x = g('w1x') * ln2w[:, None, :]
    w2 = g('w2') * lnmw[:, None, :]

    # [L, out, in] -> transpose -> [L, in, out] -> [L, ktiles, 128, out]
    # -> partition-major [L, 128, ktiles*out]
    def pm(w, ktiles, out_cols):
        wt = w[:layers].transpose(0, 2, 1).reshape(layers, ktiles, 128, out_cols)
        return np.ascontiguousarray(wt.transpose(0, 2, 1, 3)
                                    .reshape(layers, 128, ktiles * out_cols)
                                    ).astype(np.float16)

    WQt, WKt, WVt, WOt = (pm(w, KD, DIM) for w in (wq, wk, wv, wo))
    W2t = pm(w2, KH, DIM)
    # G/U: [L, in, hidden] -> [L, 4, 128part, 6ktiles*512]
    def pmg(w):
        wt = w[:layers].transpose(0, 2, 1)                  # [L, 768, 2048]
        wt = wt.reshape(layers, KD, 128, 4, 512)
        wt = wt.transpose(0, 3, 2, 1, 4)                    # [L, 4, 128, 6, 512]
        return np.ascontiguousarray(wt.reshape(layers, 4, 128, KD * 512)
                                    ).astype(np.float16)
    W1Gt, W1Xt = pmg(w1g), pmg(w1x)

    sinp = np.ascontiguousarray(rope[:, :HD].T)
    cosp = np.ascontiguousarray(rope[:, HD:].T)
    SINt = np.tile(sinp, (2, BPC)).astype(np.float16)
    COSt = np.tile(cosp, (2, BPC)).astype(np.float16)

    p64 = np.zeros((64, 64), np.float32)
    for i2 in range(32):
        p64[2 * i2 + 1, 2 * i2] = -1.0
        p64[2 * i2, 2 * i2 + 1] = 1.0
    PERMt = np.zeros((128, 128), np.float32)
    PERMt[0:64, 0:64] = p64
    PERMt[64:128, 64:128] = p64
    PERMt = PERMt.astype(np.float16)

    REP2t = np.zeros((33, 128), np.float16)
    REP2t[0, 0:64] = 1.0
    REP2t[32, 64:128] = 1.0

    xp = x + pos
    in_maps = []
    for c in range(NCORES):
        xc = xp[BPC * c:BPC * (c + 1)].reshape(T, DIM).T
        in_maps.append({
            "x_fm": np.ascontiguousarray(xc),
            "WQ": WQt, "WK": WKt, "WV": WVt, "WO": WOt,
            "W1G": W1Gt, "W1X": W1Xt, "W2": W2t,
            "ONES": np.ones((128, 128), np.float16),
            "SIN": SINt, "COS": COSt, "PERM": PERMt, "REP2": REP2t,
        })
    return in_maps


def kernel(_layers=DEPTH, _trace=False, _dbg=False, **inputs):
    _install_ntff_shim()
    from concourse import bass_utils
    key = (_layers, _dbg)
    if key not in _CACHE:
        _CACHE[key] = _build(_layers, dbg=_dbg)
    nc = _CACHE[key]
    in_maps = _prep_host(inputs, _layers)
    res = bass_utils.run_bass_kernel_spmd(nc, in_maps, core_ids=list(range(NCORES)),
                                          trace=_trace)
    out = np.empty((B, NTOK, DIM), np.float32)
    for c in range(NCORES):
        o = res.results[c]["out_fm"]
        out[BPC * c:BPC * (c + 1)] = o.T.reshape(BPC, NTOK, DIM)
    kernel.last_exec_ns = res.exec_time_ns
    kernel.last_res = res
    return out


# revision 28
# speedup vs baseline: 9.2803x; 9.2803x over previous
"""Trainium2 Bass kernel for a 12-layer EVA-style ViT encoder (B=16, N=256, D=768).

Sharding: pure data-parallel over batch across 8 NeuronCores (2 images/core).
Per core: feature-major activations [feature, token] (T=512 token columns).
v2: fp16 matmuls (1 cycle/row vs f32r's 1.5), replicated LN stats (no serial
[1,T] DVE chain), softmax denominator fused into the PV matmul via a ones
column in V, batched normalization, fp32 residual stream, preloaded weights
with single large DMAs per matrix.
"""
import sys, types

sys.path.insert(0, '/opt/trn_rl_repo')

import numpy as np

B, NTOK, DIM, HEADS, HD, DEPTH, HIDDEN = 16, 256, 768, 12, 64, 12, 2048
EPS = 1e-5
NCORES = 8
BPC = B // NCORES          # batch items per core
T = BPC * NTOK             # 512 token columns per core
KD = DIM // 128            # 6
KH = HIDDEN // 128         # 16
SCALE = HD ** -0.5

_CACHE = {}


def _install_ntff_shim():
    if "antenv.axon_hooks" in sys.modules:
        return
    m = types.ModuleType("antenv.axon_hooks")
    m._hook = None
    m.set_axon_ntff_profile_hook = lambda h: setattr(m, "_hook", h)
    m.get_axon_ntff_profile_hook = lambda: m._hook
    sys.modules["antenv.axon_hooks"] = m
    try:
        from trn_agent_boot.trn_boot import _ntff_profile_via_ctypes
        m.set_axon_ntff_profile_hook(_ntff_profile_via_ctypes('/opt/axon/libaxon_pjrt.so'))
    except Exception:
        pass


def _build(layers=DEPTH, dbg=False):
    import concourse.bass as bass
    import concourse.mybir as mybir
    import concourse.tile as tile
    from concourse import bacc
    from contextlib import ExitStack

    f32 = mybir.dt.float32
    f16 = mybir.dt.float16
    AF = mybir.ActivationFunctionType
    OP = mybir.AluOpType

    nc = bacc.Bacc("TRN2", target_bir_lowering=False, debug=False)

    x_fm = nc.dram_tensor("x_fm", [DIM, T], f32, kind="ExternalInput")
    # weights, partition-major fp16: [L, 128, ktiles*outcols]
    WQ = nc.dram_tensor("WQ", [layers, 128, KD * DIM], f16, kind="ExternalInput")
    WK = nc.dram_tensor("WK", [layers, 128, KD * DIM], f16, kind="ExternalInput")
    WV = nc.dram_tensor("WV", [layers, 128, KD * DIM], f16, kind="ExternalInput")
    WO = nc.dram_tensor("WO", [layers, 128, KD * DIM], f16, kind="ExternalInput")
    W1G = nc.dram_tensor("W1G", [layers, 4, 128, KD * 512], f16, kind="ExternalInput")
    W1X = nc.dram_tensor("W1X", [layers, 4, 128, KD * 512], f16, kind="ExternalInput")
    W2 = nc.dram_tensor("W2", [layers, 128, KH * DIM], f16, kind="ExternalInput")
    SIN = nc.dram_tensor("SIN", [128, T], f16, kind="ExternalInput")
    COS = nc.dram_tensor("COS", [128, T], f16, kind="ExternalInput")
    PERM = nc.dram_tensor("PERM", [128, 128], f16, kind="ExternalInput")
    ONES = nc.dram_tensor("ONES", [128, 128], f16, kind="ExternalInput")
    REP2 = nc.dram_tensor("REP2", [33, 128], f16, kind="ExternalInput")
    out_fm = nc.dram_tensor("out_fm", [DIM, T], f32, kind="ExternalOutput")
    if dbg:
        DY = nc.dram_tensor("DY", [DIM, T], f16, kind="ExternalOutput")
        DQ = nc.dram_tensor("DQ", [DIM, T], f16, kind="ExternalOutput")
        DK = nc.dram_tensor("DK", [DIM, T], f16, kind="ExternalOutput")
        DV = nc.dram_tensor("DV", [512, 12 * 65], f16, kind="ExternalOutput")
        DE = nc.dram_tensor("DE", [128, 512], f16, kind="ExternalOutput")
        DOT = nc.dram_tensor("DOT", [DIM, T], f16, kind="ExternalOutput")
        DH = nc.dram_tensor("DH", [DIM, T], f32, kind="ExternalOutput")
        DS = nc.dram_tensor("DS", [HIDDEN, T], f16, kind="ExternalOutput")

    with tile.TileContext(nc) as tc:
        with ExitStack() as ctx:
            ctx.enter_context(nc.allow_low_precision(
                reason="fp16 matmul inputs, fp32 psum accumulation and residual"))
            const = ctx.enter_context(tc.tile_pool(name="const", bufs=1))
            hp = ctx.enter_context(tc.tile_pool(name="hp", bufs=1))
            stp = ctx.enter_context(tc.tile_pool(name="stp", bufs=1))   # stats f16 tiles
            lnp = ctx.enter_context(tc.tile_pool(name="lnp", bufs=1))   # LN chain tiles
            yp = ctx.enter_context(tc.tile_pool(name="yp", bufs=1))
            qrawp = ctx.enter_context(tc.tile_pool(name="qrawp", bufs=1))
            ropep = ctx.enter_context(tc.tile_pool(name="ropep", bufs=1))
            rtmp = ctx.enter_context(tc.tile_pool(name="rtmp", bufs=2))
            vp = ctx.enter_context(tc.tile_pool(name="vp", bufs=1))
            ep = ctx.enter_context(tc.tile_pool(name="ep", bufs=1))
            dnp = ctx.enter_context(tc.tile_pool(name="dnp", bufs=1))
            op_ = ctx.enter_context(tc.tile_pool(name="op", bufs=1))
            sp = ctx.enter_context(tc.tile_pool(name="sp", bufs=1))
            snp = ctx.enter_context(tc.tile_pool(name="snp", bufs=1))
            wqkp = ctx.enter_context(tc.tile_pool(name="wqkp", bufs=1))
            wvop = ctx.enter_context(tc.tile_pool(name="wvop", bufs=1))
            wmp = ctx.enter_context(tc.tile_pool(name="wmp", bufs=2))
            w2p = ctx.enter_context(tc.tile_pool(name="w2p", bufs=1))
            PS = ctx.enter_context(tc.tile_pool(name="PS", bufs=1, space="PSUM"))

            def pst(tag, shape=None, name=None):
                return PS.tile(shape or [128, T], f32,
                               name=name or f"ps_{tag}_{nc.next_id()}", tag=tag)

            # ---------------- constants ----------------
            ones16 = const.tile([128, 128], f16)
            nc.sync.dma_start(out=ones16, in_=ONES[:, :])
            sin16 = const.tile([128, T], f16)
            nc.sync.dma_start(out=sin16, in_=SIN[:, :])
            cos16 = const.tile([128, T], f16)
            nc.sync.dma_start(out=cos16, in_=COS[:, :])
            perm16 = const.tile([128, 128], f16)
            nc.sync.dma_start(out=perm16, in_=PERM[:, :])
            rep2 = const.tile([33, 128], f16)
            nc.sync.dma_start(out=rep2, in_=REP2[:, :])
            epsc = const.tile([128, 1], f32)
            nc.vector.memset(epsc, EPS)
            warm8 = const.tile([128, 8], f16)
            nc.vector.memset(warm8, 1.0)

            # PE warmup
            wps = pst("p7", [8, 8], name="warmps")
            nc.tensor.matmul(wps, warm8[:, :], warm8[:, 0:8], start=True, stop=True)

            # V tiles (token-major, 12 heads x (64 cols + 1 ones col))
            vtm = []
            for mt in range(4):
                v_ = vp.tile([128, 12, 65], f16, name=f"vtm{mt}", tag=f"v{mt}")
                nc.vector.memset(v_[:, :, 64:65], 1.0)
                vtm.append(v_)
            # softmax denominator tiles: rows 0 and 32 hold the two heads of a
            # group (32-aligned partition shifts from PSUM row 64); other rows
            # stay 1.0 so Ln/Exp of them is benign
            den65 = dnp.tile([33, T], f32, name="den65", tag="den")
            nc.vector.memset(den65, 1.0)
            lnd65 = dnp.tile([33, T], f32, name="lnd65", tag="lnd")
            rec65 = dnp.tile([33, T], f16, name="rec65", tag="rec")

            h = []
            for k in range(KD):
                t_ = hp.tile([128, T], f32, name=f"h_{k}", tag=f"h{k}")
                nc.sync.dma_start(out=t_, in_=x_fm[128 * k:128 * (k + 1), :])
                h.append(t_)

            def layer_norm(tag, src_tiles, D, n_src):
                """Replicated-stats LN. src_tiles: list of [128,T] f32 tiles (or f16).
                Returns (A16, bneg16): replicated [128,T] f16 rstd and -mean."""
                inv = 1.0 / D
                srowR = pst("p6", name=f"srow_{tag}")
                qrowR = pst("p7", name=f"qrow_{tag}")
                for k in range(n_src):
                    h16 = stp.tile([128, T], f16, name=f"h16_{tag}_{k}", tag=f"st{k % 3}")
                    nc.scalar.copy(h16[:, :], src_tiles[k][:, :])
                    sq16 = stp.tile([128, T], f16, name=f"sq16_{tag}_{k}", tag=f"sq{k % 3}")
                    nc.scalar.activation(sq16[:, :], src_tiles[k][:, :], AF.Square)
                    nc.tensor.matmul(srowR, ones16[:, :], h16[:, :],
                                     start=(k == 0), stop=(k == n_src - 1))
                    nc.tensor.matmul(qrowR, ones16[:, :], sq16[:, :],
                                     start=(k == 0), stop=(k == n_src - 1))
                bneg16 = lnp.tile([128, T], f16, name=f"bneg_{tag}", tag="bneg")
                nc.vector.tensor_scalar(bneg16[:, :], srowR[:, :], -inv, None, op0=OP.mult)
                # keep the PE activity window alive through the DVE chain
                bl = pst("p6", [8, 8], name=f"blip_{tag}")
                nc.tensor.matmul(bl, warm8[:, 0:8], bneg16[:, 0:8], start=True, stop=True)
                msq = lnp.tile([128, T], f32, name=f"msq_{tag}", tag="msq")
                nc.vector.tensor_mul(msq[:, :], bneg16[:, :], bneg16[:, :])
                ve = lnp.tile([128, T], f32, name=f"ve_{tag}", tag="ve")
                nc.vector.scalar_tensor_tensor(ve[:, :], qrowR[:, :], inv, msq[:, :],
                                               op0=OP.mult, op1=OP.subtract)
                rv = lnp.tile([128, T], f32, name=f"rv_{tag}", tag="rv")
                nc.vector.reciprocal_approx_fast(out=rv[:, :], in_=ve[:, :])
                A16 = lnp.tile([128, T], f16, name=f"A_{tag}", tag="A16")
                nc.scalar.activation(A16[:, :], rv[:, :], AF.Sqrt)
                bl2 = pst("p6", [8, 8], name=f"blip2_{tag}")
                nc.tensor.matmul(bl2, warm8[:, 0:8], A16[:, 0:8], start=True, stop=True)
                return A16, bneg16

            def ln_apply(tag, k, src, A16, bneg16, out_pool, out_tag):
                tmp = rtmp.tile([128, T], f16, name=f"lt_{tag}_{k}", tag="lnt")
                nc.vector.tensor_add(tmp[:, :], src[:, :], bneg16[:, :])
                y_ = out_pool.tile([128, T], f16, name=f"y_{tag}_{k}", tag=out_tag)
                nc.vector.tensor_mul(y_[:, :], tmp[:, :], A16[:, :])
                return y_

            for l in range(layers):
                # prefetch W2 for this layer early (used last)
                w2_all = w2p.tile([128, KH, DIM], f16, name=f"w2_{l}", tag="w2")
                nc.sync.dma_start(out=w2_all, in_=W2[l, :, :])

                # ---------------- LN1 ----------------
                A1, B1 = layer_norm(f"l1_{l}", h, DIM, KD)
                y1 = [ln_apply(f"y1_{l}", k, h[k], A1, B1, yp, f"y{k}")
                      for k in range(KD)]

                # ---------------- Q,K projections + RoPE (m-outer) ----------------
                wq_all = wqkp.tile([128, KD, DIM], f16, name=f"wq_{l}", tag="wq")
                nc.sync.dma_start(out=wq_all, in_=WQ[l, :, :])
                wk_all = wqkp.tile([128, KD, DIM], f16, name=f"wk_{l}", tag="wk")
                nc.sync.dma_start(out=wk_all, in_=WK[l, :, :])
                QK_TAGS = ["p0", "p1", "p2", "p3"]
                ROT_TAGS = ["p4", "p5"]
                qs, ks = [], []
                for m in range(KD):
                    # Q and K for the same m interleaved across two PSUM banks
                    psq = pst(QK_TAGS[(2 * m) % 4], name=f"psq{l}_{m}")
                    psk = pst(QK_TAGS[(2 * m + 1) % 4], name=f"psk{l}_{m}")
                    for k in range(KD):
                        nc.tensor.matmul(psq, wq_all[:, k, 128 * m:128 * (m + 1)],
                                         y1[k][:, :], start=(k == 0), stop=(k == KD - 1))
                        nc.tensor.matmul(psk, wk_all[:, k, 128 * m:128 * (m + 1)],
                                         y1[k][:, :], start=(k == 0), stop=(k == KD - 1))
                    for wname, ps, outs in (("q", psq, qs), ("k", psk, ks)):
                        idx = (0 if wname == "q" else 1) + 2 * m
                        raw = qrawp.tile([128, T], f16, name=f"{wname}raw{l}_{m}",
                                         tag=f"qr{idx % 2}")
                        nc.scalar.copy(raw[:, :], ps[:, :])
                        rot = pst(ROT_TAGS[idx % 2], name=f"rot{wname}{l}_{m}")
                        nc.tensor.matmul(rot, perm16[:, :], raw[:, :], start=True, stop=True)
                        t1 = rtmp.tile([128, T], f16, name=f"t1{wname}{l}_{m}", tag="t1")
                        nc.vector.tensor_mul(t1[:, :], rot[:, :], sin16[:, :])
                        t2 = rtmp.tile([128, T], f16, name=f"t2{wname}{l}_{m}", tag="t2")
                        nc.vector.tensor_mul(t2[:, :], raw[:, :], cos16[:, :])
                        rp = ropep.tile([128, T], f16, name=f"{wname}p{l}_{m}",
                                        tag=f"{wname}p{m}")
                        nc.vector.tensor_add(rp[:, :], t1[:, :], t2[:, :])
                        outs.append(rp)

                # ---------------- V projection (token-major, ones col persists) ----
                wv_all = wvop.tile([128, KD, DIM], f16, name=f"wv_{l}", tag="wv")
                nc.sync.dma_start(out=wv_all, in_=WV[l, :, :])
                V_TAGS = ["p0", "p1", "p2", "p3"]
                for mt in range(4):
                    psvA = pst(V_TAGS[(2 * mt) % 4], [128, 6, 64], name=f"psvA{l}_{mt}")
                    psvB = pst(V_TAGS[(2 * mt + 1) % 4], [128, 6, 64], name=f"psvB{l}_{mt}")
                    for k in range(KD):
                        nc.tensor.matmul(psvA, y1[k][:, 128 * mt:128 * (mt + 1)],
                                         wv_all[:, k, 0:384], start=(k == 0), stop=(k == KD - 1))
                        nc.tensor.matmul(psvB, y1[k][:, 128 * mt:128 * (mt + 1)],
                                         wv_all[:, k, 384:768], start=(k == 0), stop=(k == KD - 1))
                    # scatter heads into 65-stride slots (col 64 of each head = ones)
                    nc.scalar.copy(vtm[mt][:, 0:6, 0:64], psvA[:, :, :])
                    nc.scalar.copy(vtm[mt][:, 6:12, 0:64], psvB[:, :, :])

                if dbg and l == 0:
                    for k in range(KD):
                        nc.sync.dma_start(out=DY[128 * k:128 * (k + 1), :], in_=y1[k][:, :])
                        nc.sync.dma_start(out=DQ[128 * k:128 * (k + 1), :], in_=qs[k][:, :])
                        nc.sync.dma_start(out=DK[128 * k:128 * (k + 1), :], in_=ks[k][:, :])
                    for mt in range(4):
                        nc.sync.dma_start(out=DV[128 * mt:128 * (mt + 1), :],
                                          in_=vtm[mt][:, :, :])

                # ---------------- attention ----------------
                o16 = [op_.tile([128, T], f16, name=f"o16_{l}_{p}", tag=f"o{p}")
                       for p in range(KD)]
                for p in range(KD):          # head-pair groups
                    psO = {}
                    PO_TAGS = ["p2", "p3"] if p % 2 == 0 else ["p0", "p1"]
                    for jj, (hh, i) in enumerate(
                            [(2 * p, 0), (2 * p, 1), (2 * p + 1, 0), (2 * p + 1, 1)]):
                        off = 64 * (hh % 2)
                        psS = pst(["p4", "p5"][jj % 2], name=f"psS{l}_{p}_{jj}")
                        for kt in range(2):
                            nc.tensor.matmul(
                                psS[:, NTOK * kt:NTOK * (kt + 1)],
                                ks[p][off:off + 64,
                                      256 * i + 128 * kt:256 * i + 128 * (kt + 1)],
                                qs[p][off:off + 64, 256 * i:256 * (i + 1)],
                                start=True, stop=True)
                        eT = ep.tile([128, 2 * NTOK], f16, name=f"eT{l}_{p}_{jj}",
                                     tag=f"eT{(4 * p + jj) % 4}")
                        nc.scalar.activation(eT[:, :], psS[:, :], AF.Exp)
                        if dbg and l == 0 and p == 0 and jj == 0:
                            nc.sync.dma_start(out=DE[:, :], in_=eT[:, :])
                        if hh not in psO:
                            psO[hh] = pst(PO_TAGS[hh % 2], [65, 2 * NTOK],
                                          name=f"psO{l}_{p}_{hh}")
                        for kt in range(2):
                            nc.tensor.matmul(psO[hh][:, NTOK * i:NTOK * (i + 1)],
                                             vtm[2 * i + kt][:, hh, :],
                                             eT[:, NTOK * kt:NTOK * (kt + 1)],
                                             start=(kt == 0), stop=(kt == 1))
                        r0 = 32 * (hh % 2)
                        nc.scalar.copy(den65[r0:r0 + 1, NTOK * i:NTOK * (i + 1)],
                                       psO[hh][64:65, NTOK * i:NTOK * (i + 1)])
                    # rec = 1/den via fast-approx reciprocal on DVE (no act table)
                    nc.vector.reciprocal_approx_fast(out=lnd65[:, :], in_=den65[:, :])
                    nc.scalar.copy(rec65[:, :], lnd65[:, :])
                    psR = pst(["p6", "p7"][p % 2], name=f"psR{l}_{p}")
                    nc.tensor.matmul(psR, rep2[:, :], rec65[:, :], start=True, stop=True)
                    Rec16 = dnp.tile([128, T], f16, name=f"Rec{l}_{p}", tag="Rec")
                    nc.scalar.copy(Rec16[:, :], psR[:, :])
                    for hh in (2 * p, 2 * p + 1):
                        off = 64 * (hh % 2)
                        nc.vector.tensor_mul(o16[p][off:off + 64, :],
                                             psO[hh][0:64, :], Rec16[off:off + 64, :])

                # ---------------- O projection + residual ----------------
                wo_all = wvop.tile([128, KD, DIM], f16, name=f"wo_{l}", tag="wo")
                nc.sync.dma_start(out=wo_all, in_=WO[l, :, :])
                O_TAGS = ["p5", "p6", "p7", "p0"]
                for m0 in range(0, KD, 2):
                    psa = pst(O_TAGS[m0 % 4], name=f"psh{l}_{m0}")
                    psb = pst(O_TAGS[(m0 + 1) % 4], name=f"psh{l}_{m0 + 1}")
                    for k in range(KD):
                        nc.tensor.matmul(psa, wo_all[:, k, 128 * m0:128 * (m0 + 1)],
                                         o16[k][:, :], start=(k == 0), stop=(k == KD - 1))
                        nc.tensor.matmul(psb, wo_all[:, k, 128 * (m0 + 1):128 * (m0 + 2)],
                                         o16[k][:, :], start=(k == 0), stop=(k == KD - 1))
                    nc.vector.tensor_add(h[m0][:, :], h[m0][:, :], psa[:, :])
                    nc.vector.tensor_add(h[m0 + 1][:, :], h[m0 + 1][:, :], psb[:, :])

                if dbg and l == 0:
                    for k in range(KD):
                        nc.sync.dma_start(out=DOT[128 * k:128 * (k + 1), :], in_=o16[k][:, :])
                        nc.sync.dma_start(out=DH[128 * k:128 * (k + 1), :], in_=h[k][:, :])

                # ---------------- LN2 ----------------
                A2, B2 = layer_norm(f"l2_{l}", h, DIM, KD)
                y2 = [ln_apply(f"y2_{l}", k, h[k], A2, B2, yp, f"y{k}")
                      for k in range(KD)]

                # ---------------- MLP G/U (chunks of 128 hidden) ----------------
                srow2 = pst("p6", name=f"srowm_{l}")
                qrow2 = pst("p7", name=f"qrowm_{l}")
                G_TAGS = ["p1", "p2", "p3"]
                U_TAGS = ["p4", "p5", "p0"]
                s_list = []
                for c4 in range(4):
                    wg4 = wmp.tile([128, KD, 512], f16, name=f"wg{l}_{c4}", tag="wg")
                    nc.sync.dma_start(out=wg4, in_=W1G[l, c4, :, :])
                    wx4 = wmp.tile([128, KD, 512], f16, name=f"wx{l}_{c4}", tag="wx")
                    nc.sync.dma_start(out=wx4, in_=W1X[l, c4, :, :])
                    for cc in range(4):
                        c = 4 * c4 + cc
                        psG = pst(G_TAGS[c % 3], name=f"psG{l}_{c}")
                        psU = pst(U_TAGS[c % 3], name=f"psU{l}_{c}")
                        for k in range(KD):
                            nc.tensor.matmul(psG, wg4[:, k, 128 * cc:128 * (cc + 1)],
                                             y2[k][:, :], start=(k == 0), stop=(k == KD - 1))
                            nc.tensor.matmul(psU, wx4[:, k, 128 * cc:128 * (cc + 1)],
                                             y2[k][:, :], start=(k == 0), stop=(k == KD - 1))
                        # 2*silu(g)*u = (tanh(g/2)+1)*(g*u); the factor of 2 is
                        # washed out by the inner LayerNorm
                        th = sp.tile([128, T], f16, name=f"th{l}_{c}", tag=f"th{c % 2}")
                        nc.scalar.activation(th[:, :], psG[:, :], AF.Tanh, scale=0.5)
                        uc = sp.tile([128, T], f16, name=f"uc{l}_{c}", tag=f"uc{c % 2}")
                        nc.scalar.copy(uc[:, :], psU[:, :])
                        pp = sp.tile([128, T], f16, name=f"pp{l}_{c}", tag=f"pp{c % 2}")
                        nc.vector.tensor_mul(pp[:, :], psG[:, :], uc[:, :])
                        s_ = snp.tile([128, T], f16, name=f"s{l}_{c}", tag=f"s{c}")
                        nc.vector.scalar_tensor_tensor(s_[:, :], th[:, :], 1.0, pp[:, :],
                                                       op0=OP.add, op1=OP.mult)
                        sq_ = stp.tile([128, T], f16, name=f"ssq{l}_{c}", tag=f"sq{c % 3}")
                        nc.scalar.activation(sq_[:, :], s_[:, :], AF.Square)
                        nc.tensor.matmul(srow2, ones16[:, :], s_[:, :],
                                         start=(c == 0), stop=(c == KH - 1))
                        nc.tensor.matmul(qrow2, ones16[:, :], sq_[:, :],
                                         start=(c == 0), stop=(c == KH - 1))
                        s_list.append(s_)

                if dbg and l == 0:
                    for c in range(KH):
                        nc.sync.dma_start(out=DS[128 * c:128 * (c + 1), :], in_=s_list[c][:, :])

                # ---------------- MLP LN ----------------
                inv = 1.0 / HIDDEN
                bnegm = lnp.tile([128, T], f16, name=f"bnegm_{l}", tag="bneg")
                nc.vector.tensor_scalar(bnegm[:, :], srow2[:, :], -inv, None, op0=OP.mult)
                blm = pst("p6", [8, 8], name=f"blipm_{l}")
                nc.tensor.matmul(blm, warm8[:, 0:8], bnegm[:, 0:8], start=True, stop=True)
                msqm = lnp.tile([128, T], f32, name=f"msqm_{l}", tag="msq")
                nc.vector.tensor_mul(msqm[:, :], bnegm[:, :], bnegm[:, :])
                vem = lnp.tile([128, T], f32, name=f"vem_{l}", tag="ve")
                nc.vector.scalar_tensor_tensor(vem[:, :], qrow2[:, :], inv, msqm[:, :],
                                               op0=OP.mult, op1=OP.subtract)
                rvm = lnp.tile([128, T], f32, name=f"rvm_{l}", tag="rv")
                nc.vector.reciprocal_approx_fast(out=rvm[:, :], in_=vem[:, :])
                Am = lnp.tile([128, T], f16, name=f"Am_{l}", tag="A16")
                nc.scalar.activation(Am[:, :], rvm[:, :], AF.Sqrt)
                blm2 = pst("p6", [8, 8], name=f"blipm2_{l}")
                nc.tensor.matmul(blm2, warm8[:, 0:8], Am[:, 0:8], start=True, stop=True)
                mN = []
                for c in range(KH):
                    tmp = rtmp.tile([128, T], f16, name=f"mt{l}_{c}", tag="lnt")
                    nc.vector.tensor_add(tmp[:, :], s_list[c][:, :], bnegm[:, :])
                    nc.vector.tensor_mul(s_list[c][:, :], tmp[:, :], Am[:, :])
                    mN.append(s_list[c])

                # ---------------- W2 + residual ----------------
                W2_TAGS = ["p1", "p2", "p3", "p4"]
                for m0 in range(0, KD, 2):
                    psa = pst(W2_TAGS[m0 % 4], name=f"psm{l}_{m0}")
                    psb = pst(W2_TAGS[(m0 + 1) % 4], name=f"psm{l}_{m0 + 1}")
                    for k in range(KH):
                        nc.tensor.matmul(psa, w2_all[:, k, 128 * m0:128 * (m0 + 1)],
                                         mN[k][:, :], start=(k == 0), stop=(k == KH - 1))
                        nc.tensor.matmul(psb, w2_all[:, k, 128 * (m0 + 1):128 * (m0 + 2)],
                                         mN[k][:, :], start=(k == 0), stop=(k == KH - 1))
                    nc.vector.tensor_add(h[m0][:, :], h[m0][:, :], psa[:, :])
                    nc.vector.tensor_add(h[m0 + 1][:, :], h[m0 + 1][:, :], psb[:, :])

            for k in range(KD):
                nc.sync.dma_start(out=out_fm[128 * k:128 * (k + 1), :],
                                  in_=h[k][:, :])

    nc.compile()
    return nc


def _prep_host(inputs, layers=DEPTH):
    x = np.asarray(inputs['x'], np.float32)
    pos = np.asarray(inputs['pos_embed'], np.float32)
    rope = np.asarray(inputs['rope_emb'], np.float32)
    g = lambda n: np.asarray(inputs[n], np.float32)

    for n in ('bq', 'bv', 'bo', 'b1g', 'b1x', 'b2', 'ln1_b', 'ln2_b', 'lnm_b'):
        assert np.abs(g(n)).max() == 0.0, f"nonzero bias {n} unsupported"

    ln1w, ln2w, lnmw = g('ln1_w'), g('ln2_w'), g('lnm_w')
    wq = g('wq') * ln1w[:, None, :] * SCALE
    wk = g('wk') * ln1w[:, None, :]
    wv = g('wv') * ln1w[:, None, :]
    wo = g('wo')
    w1g = g('w1g') * ln2w[:, None, :]
    w1x = g('w1x') * ln2w[:, None, :]
    w2 = g('w2') * lnmw[:, None, :]

    # [L, out, in] -> transpose -> [L, in, out] -> [L, ktiles, 128, out]
    # -> partition-major [L, 128, ktiles*out]
    def pm(w, ktiles, out_cols):
        wt = w[:layers].transpose(0, 2, 1).reshape(layers, ktiles, 128, out_cols)
        return np.ascontiguousarray(wt.transpose(0, 2, 1, 3)
                                    .reshape(layers, 128, ktiles * out_cols)
                                    ).astype(np.float16)

    WQt, WKt, WVt, WOt = (pm(w, KD, DIM) for w in (wq, wk, wv, wo))
    W2t = pm(w2, KH, DIM)
    # G/U: [L, in, hidden] -> [L, 4, 128part, 6ktiles*512]
    def pmg(w):
        wt = w[:layers].transpose(0, 2, 1)                  # [L, 768, 2048]
        wt = wt.reshape(layers, KD, 128, 4, 512)
        wt = wt.transpose(0, 3, 2, 1, 4)                    # [L, 4, 128, 6, 512]
        return np.ascontiguousarray(wt.reshape(layers, 4, 128, KD * 512)
                                    ).astype(np.float16)
    W1Gt, W1Xt = pmg(w1g), pmg(w1x)

    sinp = np.ascontiguousarray(rope[:, :HD].T)
    cosp = np.ascontiguousarray(rope[:, HD:].T)
    SINt = np.tile(sinp, (2, BPC)).astype(np.float16)
    COSt = np.tile(cosp, (2, BPC)).astype(np.float16)

    p64 = np.zeros((64, 64), np.float32)
    for i2 in range(32):
        p64[2 * i2 + 1, 2 * i2] = -1.0
        p64[2 * i2, 2 * i2 + 1] = 1.0
    PERMt = np.zeros((128, 128), np.float32)
    PERMt[0:64, 0:64] = p64
    PERMt[64:128, 64:128] = p64
    PERMt = PERMt.astype(np.float16)

    REP2t = np.zeros((33, 128), np.float16)
    REP2t[0, 0:64] = 1.0
    REP2t[32, 64:128] = 1.0

    xp = x + pos
    in_maps = []
    for c in range(NCORES):
        xc = xp[BPC * c:BPC * (c + 1)].reshape(T, DIM).T
        in_maps.append({
            "x_fm": np.ascontiguousarray(xc),
            "WQ": WQt, "WK": WKt, "WV": WVt, "WO": WOt,
            "W1G": W1Gt, "W1X": W1Xt, "W2": W2t,
            "ONES": np.ones((128, 128), np.float16),
            "SIN": SINt, "COS": COSt, "PERM": PERMt, "REP2": REP2t,
        })
    return in_maps


def kernel(_layers=DEPTH, _trace=False, _dbg=False, **inputs):
    _install_ntff_shim()
    from concourse import bass_utils
    key = (_layers, _dbg)
    if key not in _CACHE:
        _CACHE[key] = _build(_layers, dbg=_dbg)
    nc = _CACHE[key]
    in_maps = _prep_host(inputs, _layers)
    res = bass_utils.run_bass_kernel_spmd(nc, in_maps, core_ids=list(range(NCORES)),
                                          trace=_trace)
    out = np.empty((B, NTOK, DIM), np.float32)
    for c in range(NCORES):
        o = res.results[c]["out_fm"]
        out[BPC * c:BPC * (c + 1)] = o.T.reshape(BPC, NTOK, DIM)
    kernel.last_exec_ns = res.exec_time_ns
    kernel.last_res = res
    return out
